# revision 16
# baseline (speedup 1.0000x reference)
"""Trainium2 Bass kernel for nn_AttentionBlock_56075093016781 (8 NeuronCores, SPMD).

Reference semantics (b=2, c=512, L=1024, num_heads=8):
  xn  = batchnorm(x) (stats over batch+length per channel) * gamma + beta
  qkv = w_qkv @ xn + b_qkv                  (1x1 conv over channels)
  layout quirk: qkv -> (b, 3*nh, hd, L) -> (b, hd, L, 3*nh); split q,k,v
    => 64 attention "heads" (the hd axis), feature dim 8 (the nh axis), T=1024
  w   = softmax(scale * q @ k^T) over keys, scale = (3*nh)**-0.5
  a   = w @ v ;  h[d*64+head, t] = a[head, t, d] ;  out = x + w_proj @ h + b_proj

Sharding: 8 cores = 2 batches x 4 head-groups of 16 heads. Each core computes
BN redundantly, its own q/k/v projections, attention for its 16 heads, and a
partial output projection over its 64 channels (padded to 512 rows with zero
weight rows). The host sums the 4 partials per batch and adds the residual
x + b_proj (part of the gather).

Device-side structure (v2 — PE-concurrency rewrite):
  - scores: per (quad, h-chunk of 512 queries, f-block of 128 keys), two
    [128,1024] f32 PSUM tiles each holding 2 heads; the 4 matmuls go to the
    4 distinct 32-row PE tile groups back-to-back so they stream concurrently
  - exp: tile A on ScalarE (ACTIVATE Exp), tile B on VectorE (Schraudolph
    int16 bit-trick -> bitcast bf16); selected cycles send both to ScalarE to
    absorb VectorE's extra (bias/cast/recip) work
  - AV: 4 col-group matmuls (tile_position (0,32j)) accumulate into a single
    1-bank [128,512] PSUM accumulator per (quad, h); softmax denominator
    comes free from a ones-column in the v blocks
  - PSUM budget: 3x2 banks (scores) + 1 (av) + 1 (kq/v/outproj misc) = 8
  - kq projections for quad qd+1 and the v projection are interleaved into
    the attention cycles of quad qd / quad 0
  - ACT exp-table preload + PE HAM-warmup dummy matmuls run during BN
  - BN rstd via rsqrt bit-trick + 2 Newton steps, batched over all 4
    channel blocks; x shipped as bf16
"""
import numpy as np
import ml_dtypes

import concourse.bass as bass
import concourse.bacc as bacc
import concourse.mybir as mybir
import concourse.tile as tile
from concourse.bass_utils import run_bass_kernel_spmd

F32 = mybir.dt.float32
BF16 = mybir.dt.bfloat16
I16 = mybir.dt.int16
I32 = mybir.dt.int32

B, C, L = 2, 512, 1024
NH = 8          # feature dim of each attention head (from num_heads)
HD = 64         # number of attention heads (head_dim axis of the quirky layout)
HEADS_PER_CORE = 16
N_CORES = 8
EPS = 1e-5

# exp-assignment: cycles (of 16 per quad) where BOTH score tiles go to ScalarE
ACT_BOTH_CYCLES = (5, 13)

_CACHE = {}


def _build_nc():
    nc = bacc.Bacc(None, target_bir_lowering=False)

    # ---- DRAM I/O ----
    x2_d = nc.dram_tensor("x2", [C, 2 * L], BF16, kind="ExternalInput")      # [c, b*L]
    gamma_d = nc.dram_tensor("gamma", [C], F32, kind="ExternalInput")
    beta_d = nc.dram_tensor("beta", [C], F32, kind="ExternalInput")
    wq_d = nc.dram_tensor("wqT", [C, 512], BF16, kind="ExternalInput")       # [c, padded qch] (scale folded)
    wk_d = nc.dram_tensor("wkT", [C, 512], BF16, kind="ExternalInput")
    wv_d = nc.dram_tensor("wvT", [C, 128], BF16, kind="ExternalInput")       # [c, vch compact]
    bq_d = nc.dram_tensor("bq", [512], F32, kind="ExternalInput")            # padded, scale folded
    bk_d = nc.dram_tensor("bk", [512], F32, kind="ExternalInput")
    bv_d = nc.dram_tensor("bv", [128], F32, kind="ExternalInput")
    wp_d = nc.dram_tensor("wpT", [512, 512], BF16, kind="ExternalInput")     # [padded c, o]
    out_d = nc.dram_tensor("out", [C, L], F32, kind="ExternalOutput")
    rscr_d = nc.dram_tensor("rscr", [HEADS_PER_CORE, L], BF16)               # internal scratch (recip denoms)

    EXP_A = 184.66496
    EXP_B = 16248.75

    with tile.TileContext(nc) as tc:
        with (
            tc.tile_pool(name="singles", bufs=1) as singles,
            tc.tile_pool(name="wt", bufs=6) as wtp,
            tc.tile_pool(name="norm", bufs=3) as normp,
            tc.tile_pool(name="outp", bufs=3) as outp,
            tc.tile_pool(name="psS", bufs=3, space="PSUM") as psS,
            tc.tile_pool(name="psA", bufs=1, space="PSUM") as psA,
            tc.tile_pool(name="psM", bufs=1, space="PSUM") as psM,
        ):
            # ---- input DMAs ----
            wq = [singles.tile([128, 512], BF16, name=f"wq{i}") for i in range(4)]
            wk = [singles.tile([128, 512], BF16, name=f"wk{i}") for i in range(4)]
            wv = [singles.tile([128, 128], BF16, name=f"wv{i}") for i in range(4)]
            wp = [singles.tile([128, 512], BF16, name=f"wp{i}") for i in range(4)]
            for ct in range(4):
                nc.scalar.dma_start(wq[ct][:], wq_d[ct * 128:(ct + 1) * 128, :])
                nc.scalar.dma_start(wk[ct][:], wk_d[ct * 128:(ct + 1) * 128, :])
                nc.scalar.dma_start(wv[ct][:], wv_d[ct * 128:(ct + 1) * 128, :])

            xch = [[singles.tile([128, 1024], BF16, name=f"xc{i}_{k}") for k in range(2)]
                   for i in range(4)]
            dma_engs = [nc.sync, nc.gpsimd, nc.scalar]
            for ct in range(4):
                for k in range(2):
                    dma_engs[(2 * ct + k) % 3].dma_start(
                        xch[ct][k][:], x2_d[ct * 128:(ct + 1) * 128, k * 1024:(k + 1) * 1024])
            gam = singles.tile([128, 4], F32, name="gam")
            bet = singles.tile([128, 4], F32, name="bet")
            nc.sync.dma_start(gam[:], gamma_d.rearrange("(o p) -> p o", p=128))
            nc.sync.dma_start(bet[:], beta_d.rearrange("(o p) -> p o", p=128))
            bqt = singles.tile([128, 4], F32, name="bqt")
            bkt = singles.tile([128, 4], F32, name="bkt")
            nc.sync.dma_start(bqt[:], bq_d.rearrange("(o p) -> p o", p=128))
            nc.sync.dma_start(bkt[:], bk_d.rearrange("(o p) -> p o", p=128))
            bvb = singles.tile([128, 128], F32, name="bvb")  # bv broadcast across partitions
            nc.sync.dma_start(bvb[:], bass.AP(tensor=bv_d.tensor if hasattr(bv_d, "tensor") else bv_d,
                                              offset=0, ap=[[0, 128], [1, 128]]))

            # ---- ACT exp-table preload + vsb init (runs while DMA/BN proceed) ----
            epst = singles.tile([128, 1], F32, name="eps")
            nc.vector.memset(epst[:], EPS)
            warm = singles.tile([128, 1], BF16, name="warm")
            nc.scalar.activation(out=warm[:], in_=epst[:],
                                 func=mybir.ActivationFunctionType.Exp)

            # vsb[f, fb, head, 0:8]=v, [..,8]=1, [..,9:32]=0
            vsb = singles.tile([128, 8, 16, 32], BF16, name="vsb")
            nc.vector.memset(vsb[:, :, :, 8:32], 0.0)
            nc.vector.memset(vsb[:, :, :, 8:9], 1.0)

            # ---- BatchNorm stats (over both batches) ----
            stats = singles.tile([128, 96], F32, name="bnstats")
            sview = stats.rearrange("p (c s d) -> p c s d", c=4, s=4)
            for ct in range(4):
                for k in range(2):
                    xin = xch[ct][k].rearrange("p (s f) -> p s f", f=512)
                    for si in range(2):
                        nc.vector.bn_stats(out=sview[:, ct, 2 * k + si, :], in_=xin[:, si, :])
            mv = singles.tile([128, 4, 2], F32, name="mv")
            for ct in range(4):
                nc.vector.bn_aggr(out=mv[:, ct, :], in_=sview[:, ct, :, :])

            # ---- PE HAM warmup (dummy matmuls during BN; late ones depend on
            # the bn stats so the PE stays busy until the real kq matmuls) ----
            for i in range(6):
                psw = psM.tile([128, 512], F32, tag="m", name="warmmm")
                nc.tensor.matmul(psw[:], wq[i % 4][:, 0:128], wk[i % 4][:, 0:512],
                                 start=True, stop=True)
            statb = stats.bitcast(BF16)
            for i in range(8):
                psw = psM.tile([128, 512], F32, tag="m", name="warmm2")
                nc.tensor.matmul(psw[:], statb[:, 0:128], wk[i % 4][:, 0:512],
                                 start=True, stop=True)

            # ---- batched rstd via rsqrt bit-trick + 2 Newton iterations ----
            r4 = singles.tile([128, 4], F32, name="r4")
            nc.vector.tensor_scalar(out=r4[:], in0=mv[:, :, 1:2], scalar1=EPS,
                                    scalar2=None, op0=mybir.AluOpType.add)
            yi = singles.tile([128, 4], I32, name="yi4")
            with nc.allow_low_precision(reason="rsqrt seed bit trick"):
                nc.vector.tensor_scalar(out=yi[:], in0=r4.bitcast(I32)[:],
                                        scalar1=-1, scalar2=2 * 0x5f3759df,
                                        op0=mybir.AluOpType.mult, op1=mybir.AluOpType.add)
                nc.vector.tensor_scalar(out=yi[:], in0=yi[:], scalar1=1, scalar2=None,
                                        op0=mybir.AluOpType.logical_shift_right)
            rstd = singles.tile([128, 4], F32, name="rstd4")
            yf = yi.bitcast(F32)
            for it in range(2):
                t2 = singles.tile([128, 4], F32, name=f"t2_{it}")
                nc.vector.tensor_tensor(out=t2[:], in0=yf[:], in1=yf[:],
                                        op=mybir.AluOpType.mult)
                nc.vector.tensor_tensor(out=t2[:], in0=t2[:], in1=r4[:],
                                        op=mybir.AluOpType.mult)
                nc.vector.tensor_scalar(out=t2[:], in0=t2[:], scalar1=-0.5,
                                        scalar2=1.5, op0=mybir.AluOpType.mult,
                                        op1=mybir.AluOpType.add)
                dst = rstd if it == 1 else yi.bitcast(F32)
                nc.vector.tensor_tensor(out=dst[:], in0=yf[:], in1=t2[:],
                                        op=mybir.AluOpType.mult)
            s4 = singles.tile([128, 4], F32, name="s4")
            nc.vector.tensor_tensor(out=s4[:], in0=rstd[:], in1=gam[:],
                                    op=mybir.AluOpType.mult)
            t4 = singles.tile([128, 4], F32, name="t4")
            nc.vector.tensor_tensor(out=t4[:], in0=mv[:, :, 0:1], in1=s4[:],
                                    op=mybir.AluOpType.mult)
            nc.vector.tensor_tensor(out=t4[:], in0=bet[:], in1=t4[:],
                                    op=mybir.AluOpType.subtract)
            xn = [singles.tile([128, L], BF16, name=f"xn{i}") for i in range(4)]
            for ct in range(4):
                nc.vector.tensor_scalar(out=xn[ct][:], in0=xch[ct][0][:],
                                        scalar1=s4[:, ct:ct + 1], scalar2=t4[:, ct:ct + 1],
                                        op0=mybir.AluOpType.mult, op1=mybir.AluOpType.add)

            # ---- k/q projections (chunked; bias-add is the PSUM->SBUF move) ----
            kT = [singles.tile([128, L], BF16, name=f"kT{i}") for i in range(4)]
            qT = [singles.tile([128, L], BF16, name=f"qT{i}") for i in range(4)]

            def emit_kq_chunk(mo, which, nh_):
                wmat, bias_t, dst = (wk, bkt, kT) if which == "k" else (wq, bqt, qT)
                ps = psM.tile([128, 512], F32, tag="m", name="ps_kq")
                for kt in range(4):
                    nc.tensor.matmul(ps[:],
                                     wmat[kt][:, mo * 128:(mo + 1) * 128],
                                     xn[kt][:, nh_ * 512:(nh_ + 1) * 512],
                                     start=(kt == 0), stop=(kt == 3))
                nc.vector.tensor_scalar(out=dst[mo][:, nh_ * 512:(nh_ + 1) * 512],
                                        in0=ps[:], scalar1=bias_t[:, mo:mo + 1],
                                        scalar2=None, op0=mybir.AluOpType.add)

            def emit_v(tt):
                ps_full = psM.tile([128, 512], F32, tag="m", name="psv")
                ps = ps_full[:, 0:128]
                for kt in range(4):
                    nc.tensor.matmul(ps[:], xn[kt][:, tt * 128:(tt + 1) * 128],
                                     wv[kt][:], start=(kt == 0), stop=(kt == 3))
                nc.vector.tensor_tensor(
                    out=vsb[:, tt, :, 0:8],
                    in0=ps.rearrange("p (h d) -> p h d", d=8),
                    in1=bvb.rearrange("p (h d) -> p h d", d=8),
                    op=mybir.AluOpType.add)

            KQ_ORDER = [("k", 0), ("q", 0), ("k", 1), ("q", 1)]
            emit_kq_chunk(0, "k", 0)
            emit_kq_chunk(0, "q", 0)
            emit_v(0)
            emit_v(1)

            # ---- attention: 4 quads x (2 h-chunks x 8 f-blocks) ----
            hout = [singles.tile([128, L], BF16, name=f"ho{i}") for i in range(4)]

            def emit_exp(ps, use_act):
                if use_act:
                    wt = wtp.tile([128, 1024], BF16, tag="wt")
                    nc.scalar.activation(out=wt[:], in_=ps[:],
                                         func=mybir.ActivationFunctionType.Exp)
                else:
                    wti = wtp.tile([128, 1024], I16, tag="wt")
                    with nc.allow_low_precision(reason="schraudolph exp approx, validated"):
                        nc.vector.tensor_scalar(
                            out=wti[:], in0=ps[:], scalar1=EXP_A, scalar2=EXP_B,
                            op0=mybir.AluOpType.mult, op1=mybir.AluOpType.add)
                    wt = wti.bitcast(BF16)
                return wt

            rscr_t = rscr_d.tensor if hasattr(rscr_d, "tensor") else rscr_d

            def emit_norm(qd, h, a_sb, rb, mult_dve):
                """Recip the denominators of half h and scale a_sb into hout."""
                hc = slice(h * 512, (h + 1) * 512)
                dt = normp.tile([32, 64], BF16, tag="dt", name="dt")
                for j in range(4):
                    eng = nc.sync if j % 2 == 0 else nc.gpsimd
                    eng.dma_start(
                        dt[8 * j:8 * j + 8, :],
                        a_sb[32 * j + 8:32 * j + 9, hc].rearrange("p (s f) -> p s f", f=64))
                rt = normp.tile([32, 64], BF16, tag="rt", name="rt")
                with nc.allow_low_precision(reason="bf16 softmax denom recip, validated"):
                    nc.vector.reciprocal(out=rt[:], in_=dt[:])
                for j in range(4):
                    hd_ = 4 * qd + j
                    eng = nc.sync if j % 2 == 0 else nc.gpsimd
                    eng.dma_start(
                        rscr_d[hd_, h * 512:(h + 1) * 512].rearrange("(s f) -> s f", f=64),
                        rt[8 * j:8 * j + 8, :])
                for j in range(4):
                    hd_ = 4 * qd + j
                    eng = nc.sync if j % 2 == 0 else nc.gpsimd
                    eng.dma_start(
                        rb[32 * j:32 * j + 32, hc],
                        bass.AP(tensor=rscr_t, offset=hd_ * L + h * 512,
                                ap=[[0, 32], [1, 512]]))
                mul_eng = nc.vector if mult_dve else nc.gpsimd
                mul_eng.tensor_tensor(out=hout[qd][:, hc], in0=a_sb[:, hc], in1=rb[:, hc],
                                      op=mybir.AluOpType.mult)

            def emit_outproj(mo, th, pool, tag, use_act, dma_eng):
                ps = pool.tile([128, 512], F32, tag=tag, name="pso")
                for qdd in range(4):
                    nc.tensor.matmul(ps[:], wp[qdd][:, mo * 128:(mo + 1) * 128],
                                     hout[qdd][:, th * 512:(th + 1) * 512],
                                     start=(qdd == 0), stop=(qdd == 3))
                ot = outp.tile([128, 512], F32, tag="ot")
                if use_act:
                    nc.scalar.activation(out=ot[:], in_=ps[:],
                                         func=mybir.ActivationFunctionType.Copy)
                else:
                    nc.vector.tensor_copy(ot[:], ps[:])
                dma_eng.dma_start(out_d[mo * 128:(mo + 1) * 128, th * 512:(th + 1) * 512],
                                  ot[:])

            for qd in range(4):
                if qd == 1:
                    for ct in range(4):
                        nc.sync.dma_start(wp[ct][:], wp_d[ct * 128:(ct + 1) * 128, :])
                a_sb = normp.tile([128, 1024], BF16, tag="asb")
                rb = normp.tile([128, 1024], BF16, tag="rb")
                av_t = {}
                pend = None  # (wtA, wtB, fb, h)

                for c in range(17):
                    if c < 16:
                        h, fb = divmod(c, 8)
                        fc = slice(fb * 128, (fb + 1) * 128)
                        hc = slice(h * 512, (h + 1) * 512)
                        A = psS.tile([128, 1024], F32, tag="sc", name="ps_scA")
                        B_ = psS.tile([128, 1024], F32, tag="sc", name="ps_scB")
                        nc.tensor.matmul(A[:, 0:512], kT[qd][0:32, fc], qT[qd][0:32, hc],
                                         tile_position=(0, 0))
                        nc.tensor.matmul(A[:, 512:1024], kT[qd][32:64, fc], qT[qd][32:64, hc],
                                         tile_position=(32, 0))
                        nc.tensor.matmul(B_[:, 0:512], kT[qd][64:96, fc], qT[qd][64:96, hc],
                                         tile_position=(64, 0))
                        nc.tensor.matmul(B_[:, 512:1024], kT[qd][96:128, fc], qT[qd][96:128, hc],
                                         tile_position=(96, 0))
                        both_act = c in ACT_BOTH_CYCLES
                        wtA = emit_exp(A, True)
                        wtB = emit_exp(B_, both_act)
                        nxt = (wtA, wtB, fb, h)
                    else:
                        nxt = None

                    if pend is not None:
                        pwtA, pwtB, pfb, ph = pend
                        if pfb == 0:
                            av_t[ph] = psA.tile([128, 512], F32, tag="av", name="av")
                        av = av_t[ph]
                        hd0 = 4 * qd
                        nc.tensor.matmul(av[0:32, :], vsb[:, pfb, hd0 + 0, :], pwtA[:, 0:512],
                                         start=(pfb == 0), stop=(pfb == 7), tile_position=(0, 0))
                        nc.tensor.matmul(av[32:64, :], vsb[:, pfb, hd0 + 1, :], pwtA[:, 512:1024],
                                         start=(pfb == 0), stop=(pfb == 7), tile_position=(0, 32))
                        nc.tensor.matmul(av[64:96, :], vsb[:, pfb, hd0 + 2, :], pwtB[:, 0:512],
                                         start=(pfb == 0), stop=(pfb == 7), tile_position=(0, 64))
                        nc.tensor.matmul(av[96:128, :], vsb[:, pfb, hd0 + 3, :], pwtB[:, 512:1024],
                                         start=(pfb == 0), stop=(pfb == 7), tile_position=(0, 96))
                        if pfb == 7:
                            # dump av(h) -> a_sb chunk (ACT for h0, DVE for h1)
                            dst = a_sb[:, ph * 512:(ph + 1) * 512]
                            if ph == 0:
                                nc.scalar.activation(out=dst, in_=av[:],
                                                     func=mybir.ActivationFunctionType.Copy)
                            else:
                                nc.vector.tensor_copy(dst, av[:])
                            emit_norm(qd, ph, a_sb, rb,
                                      mult_dve=(qd == 3 and ph == 1))
                    pend = nxt

                    # interleaved projections
                    if qd == 0:
                        if c == 0:
                            emit_v(2)
                        elif c == 1:
                            emit_kq_chunk(0, "k", 1)
                            emit_v(3)
                        elif c == 2:
                            emit_kq_chunk(0, "q", 1)
                            emit_v(4)
                        elif c in (3, 4, 5):
                            emit_v(c + 2)
                    if qd < 3 and c in (9, 11, 13, 15):
                        which, nh_ = KQ_ORDER[(c - 9) // 2]
                        emit_kq_chunk(qd + 1, which, nh_)
                    if qd == 3 and c in (13, 15):
                        mo = (c - 13) // 2
                        emit_outproj(mo, 0, psM, "m", use_act=(mo % 2 == 0),
                                     dma_eng=nc.sync if mo % 2 == 0 else nc.gpsimd)

            # ---- output projection remainder (alternate pools/engines) ----
            rem = [(2, 0), (3, 0), (0, 1), (1, 1), (2, 1), (3, 1)]
            for i, (mo, th) in enumerate(rem):
                pool, tag = (psM, "m") if i % 2 == 0 else (psA, "av")
                emit_outproj(mo, th, pool, tag, use_act=(i % 2 == 0),
                             dma_eng=nc.sync if i % 2 == 0 else nc.gpsimd)

    nc.compile()
    return nc


def _prep_inputs(x, gamma, beta, w_qkv, b_qkv, w_proj, b_proj, num_heads):
    """Shard and lay out inputs for the 8 cores."""
    nh = int(num_heads)
    hd = C // nh
    scale = (3 * nh) ** (-0.5)
    wq_full, wk_full, wv_full = w_qkv[0:C], w_qkv[C:2 * C], w_qkv[2 * C:3 * C]
    bq_full, bk_full, bv_full = b_qkv[0:C], b_qkv[C:2 * C], b_qkv[2 * C:3 * C]

    bf16 = ml_dtypes.bfloat16
    in_maps = []
    for core in range(N_CORES):
        bi = core // 4
        g = core % 4
        heads = list(range(HEADS_PER_CORE * g, HEADS_PER_CORE * (g + 1)))
        x_own = x[bi]
        x_oth = x[1 - bi]
        x2 = np.concatenate([x_own, x_oth], axis=1).astype(bf16)

        wqT = np.zeros((C, 512), np.float32)
        wkT = np.zeros((C, 512), np.float32)
        bq = np.zeros(512, np.float32)
        bk = np.zeros(512, np.float32)
        wvT = np.zeros((C, 128), np.float32)
        bv = np.zeros(128, np.float32)
        wpT = np.zeros((512, C), np.float32)
        for jl, h in enumerate(heads):
            for d in range(nh):
                ch = d * hd + h
                wqT[:, 32 * jl + d] = wq_full[ch] * scale
                wkT[:, 32 * jl + d] = wk_full[ch]
                bq[32 * jl + d] = bq_full[ch] * scale
                bk[32 * jl + d] = bk_full[ch]
                wvT[:, 8 * jl + d] = wv_full[ch]
                bv[8 * jl + d] = bv_full[ch]
                wpT[32 * jl + d, :] = w_proj[:, ch]

        in_maps.append({
            "x2": np.ascontiguousarray(x2),
            "gamma": np.ascontiguousarray(gamma.astype(np.float32)),
            "beta": np.ascontiguousarray(beta.astype(np.float32)),
            "wqT": np.ascontiguousarray(wqT.astype(bf16)),
            "wkT": np.ascontiguousarray(wkT.astype(bf16)),
            "wvT": np.ascontiguousarray(wvT.astype(bf16)),
            "bq": bq, "bk": bk, "bv": bv,
            "wpT": np.ascontiguousarray(wpT.astype(bf16)),
        })
    return in_maps


def kernel(x, gamma, beta, w_qkv, b_qkv, w_proj, b_proj, num_heads, _trace=False):
    x = np.asarray(x, dtype=np.float32)
    gamma = np.asarray(gamma, np.float32)
    beta = np.asarray(beta, np.float32)
    w_qkv = np.asarray(w_qkv, np.float32)
    b_qkv = np.asarray(b_qkv, np.float32)
    w_proj = np.asarray(w_proj, np.float32)
    b_proj = np.asarray(b_proj, np.float32)

    if "nc" not in _CACHE:
        _CACHE["nc"] = _build_nc()
    nc = _CACHE["nc"]
    in_maps = _prep_inputs(x, gamma, beta, w_qkv, b_qkv, w_proj, b_proj, num_heads)
    res = run_bass_kernel_spmd(nc, in_maps, core_ids=list(range(N_CORES)), trace=_trace)
    _CACHE["last_result"] = res

    out = np.zeros((B, C, L), np.float32)
    for bi in range(B):
        acc = x[bi] + b_proj[:, None]
        for g in range(4):
            acc = acc + np.asarray(res.results[bi * 4 + g]["out"])
        out[bi] = acc
    return out


# revision 20
# speedup vs baseline: 1.0356x; 1.0356x over previous
"""Trainium2 Bass kernel for nn_AttentionBlock_56075093016781 (8 NeuronCores, SPMD).

Reference semantics (b=2, c=512, L=1024, num_heads=8):
  xn  = batchnorm(x) (stats over batch+length per channel) * gamma + beta
  qkv = w_qkv @ xn + b_qkv                  (1x1 conv over channels)
  layout quirk: qkv -> (b, 3*nh, hd, L) -> (b, hd, L, 3*nh); split q,k,v
    => 64 attention "heads" (the hd axis), feature dim 8 (the nh axis), T=1024
  w   = softmax(scale * q @ k^T) over keys, scale = (3*nh)**-0.5
  a   = w @ v ;  h[d*64+head, t] = a[head, t, d] ;  out = x + w_proj @ h + b_proj

Sharding: 8 cores = 2 batches x 4 head-groups of 16 heads. Each core computes
BN redundantly, its own q/k/v projections, attention for its 16 heads, and a
partial output projection over its 64 channels (padded to 512 rows with zero
weight rows). The host sums the 4 partials per batch and adds the residual
x + b_proj (part of the gather).

Device-side structure (v2 — PE-concurrency rewrite):
  - scores: per (quad, h-chunk of 512 queries, f-block of 128 keys), two
    [128,1024] f32 PSUM tiles each holding 2 heads; the 4 matmuls go to the
    4 distinct 32-row PE tile groups back-to-back so they stream concurrently
  - exp: tile A on ScalarE (ACTIVATE Exp), tile B on VectorE (Schraudolph
    int16 bit-trick -> bitcast bf16); selected cycles send both to ScalarE to
    absorb VectorE's extra (bias/cast/recip) work
  - AV: 4 col-group matmuls (tile_position (0,32j)) accumulate into a single
    1-bank [128,512] PSUM accumulator per (quad, h); softmax denominator
    comes free from a ones-column in the v blocks
  - PSUM budget: 3x2 banks (scores) + 1 (av) + 1 (kq/v/outproj misc) = 8
  - kq projections for quad qd+1 and the v projection are interleaved into
    the attention cycles of quad qd / quad 0
  - ACT exp-table preload + PE HAM-warmup dummy matmuls run during BN
  - BN rstd via rsqrt bit-trick + 2 Newton steps, batched over all 4
    channel blocks; x shipped as bf16
"""
import numpy as np
import ml_dtypes

import concourse.bass as bass
import concourse.bacc as bacc
import concourse.mybir as mybir
import concourse.tile as tile
from concourse.bass_utils import run_bass_kernel_spmd

F32 = mybir.dt.float32
BF16 = mybir.dt.bfloat16
I16 = mybir.dt.int16
I32 = mybir.dt.int32

B, C, L = 2, 512, 1024
NH = 8          # feature dim of each attention head (from num_heads)
HD = 64         # number of attention heads (head_dim axis of the quirky layout)
HEADS_PER_CORE = 16
N_CORES = 8
EPS = 1e-5

# exp-assignment: cycles (of 16 per quad) where BOTH score tiles go to ScalarE
ACT_BOTH_CYCLES = (5, 13)

_CACHE = {}


def _build_nc():
    nc = bacc.Bacc(None, target_bir_lowering=False)

    # ---- DRAM I/O ----
    x2_d = nc.dram_tensor("x2", [C, 2 * L], BF16, kind="ExternalInput")      # [c, b*L]
    gamma_d = nc.dram_tensor("gamma", [C], F32, kind="ExternalInput")
    beta_d = nc.dram_tensor("beta", [C], F32, kind="ExternalInput")
    wq_d = nc.dram_tensor("wqT", [C, 512], BF16, kind="ExternalInput")       # [c, padded qch] (scale folded)
    wk_d = nc.dram_tensor("wkT", [C, 512], BF16, kind="ExternalInput")
    wv_d = nc.dram_tensor("wvT", [C, 128], BF16, kind="ExternalInput")       # [c, vch compact]
    bq_d = nc.dram_tensor("bq", [512], F32, kind="ExternalInput")            # padded, scale folded
    bk_d = nc.dram_tensor("bk", [512], F32, kind="ExternalInput")
    bv_d = nc.dram_tensor("bv", [128], F32, kind="ExternalInput")
    wp_d = nc.dram_tensor("wpT", [512, 512], BF16, kind="ExternalInput")     # [padded c, o]
    out_d = nc.dram_tensor("out", [C, L], F32, kind="ExternalOutput")
    rscr_d = nc.dram_tensor("rscr", [HEADS_PER_CORE, L], BF16)               # internal scratch (recip denoms)

    EXP_A = 184.66496
    EXP_B = 16248.75

    with tile.TileContext(nc) as tc:
        with (
            tc.tile_pool(name="singles", bufs=1) as singles,
            tc.tile_pool(name="wt", bufs=6) as wtp,
            tc.tile_pool(name="norm", bufs=3) as normp,
            tc.tile_pool(name="outp", bufs=3) as outp,
            tc.tile_pool(name="psS", bufs=3, space="PSUM") as psS,
            tc.tile_pool(name="psA", bufs=1, space="PSUM") as psA,
            tc.tile_pool(name="psM", bufs=1, space="PSUM") as psM,
        ):
            # ---- input DMAs (x2 chunks first on every queue; weights after) ----
            wq = [singles.tile([128, 512], BF16, name=f"wq{i}") for i in range(4)]
            wk = [singles.tile([128, 512], BF16, name=f"wk{i}") for i in range(4)]
            wv = [singles.tile([128, 128], BF16, name=f"wv{i}") for i in range(4)]
            wp = [singles.tile([128, 512], BF16, name=f"wp{i}") for i in range(4)]
            xch = [[singles.tile([128, 1024], BF16, name=f"xc{i}_{k}") for k in range(2)]
                   for i in range(4)]
            dma_engs = [nc.sync, nc.scalar, nc.sync, nc.scalar, nc.sync, nc.scalar,
                        nc.gpsimd, nc.gpsimd]
            for ct in range(4):
                for k in range(2):
                    dma_engs[2 * ct + k].dma_start(
                        xch[ct][k][:], x2_d[ct * 128:(ct + 1) * 128, k * 1024:(k + 1) * 1024])
            for ct in range(4):
                nc.scalar.dma_start(wq[ct][:], wq_d[ct * 128:(ct + 1) * 128, :])
                nc.sync.dma_start(wk[ct][:], wk_d[ct * 128:(ct + 1) * 128, :])
                nc.gpsimd.dma_start(wv[ct][:], wv_d[ct * 128:(ct + 1) * 128, :])
            gam = singles.tile([128, 4], F32, name="gam")
            bet = singles.tile([128, 4], F32, name="bet")
            nc.gpsimd.dma_start(gam[:], gamma_d.rearrange("(o p) -> p o", p=128))
            nc.gpsimd.dma_start(bet[:], beta_d.rearrange("(o p) -> p o", p=128))
            bqt = singles.tile([128, 4], F32, name="bqt")
            bkt = singles.tile([128, 4], F32, name="bkt")
            nc.gpsimd.dma_start(bqt[:], bq_d.rearrange("(o p) -> p o", p=128))
            nc.gpsimd.dma_start(bkt[:], bk_d.rearrange("(o p) -> p o", p=128))
            bvb = singles.tile([128, 128], F32, name="bvb")  # bv broadcast across partitions
            nc.gpsimd.dma_start(bvb[:], bass.AP(tensor=bv_d.tensor if hasattr(bv_d, "tensor") else bv_d,
                                                offset=0, ap=[[0, 128], [1, 128]]))

            # ---- ACT exp-table preload + vsb init (runs while DMA/BN proceed) ----
            epst = singles.tile([128, 1], F32, name="eps")
            nc.vector.memset(epst[:], EPS)
            warm = singles.tile([128, 1], BF16, name="warm")
            nc.scalar.activation(out=warm[:], in_=epst[:],
                                 func=mybir.ActivationFunctionType.Exp)

            # vsb[f, fb, head, 0:8]=v, [..,8]=1, [..,9:32]=0
            vsb = singles.tile([128, 8, 16, 32], BF16, name="vsb")
            nc.vector.memset(vsb[:, :, :, 8:32], 0.0)
            nc.vector.memset(vsb[:, :, :, 8:9], 1.0)

            # ---- BatchNorm stats (over both batches) ----
            stats = singles.tile([128, 96], F32, name="bnstats")
            sview = stats.rearrange("p (c s d) -> p c s d", c=4, s=4)
            for ct in range(4):
                for k in range(2):
                    xin = xch[ct][k].rearrange("p (s f) -> p s f", f=512)
                    for si in range(2):
                        nc.vector.bn_stats(out=sview[:, ct, 2 * k + si, :], in_=xin[:, si, :])
            mv = singles.tile([128, 4, 2], F32, name="mv")
            for ct in range(4):
                nc.vector.bn_aggr(out=mv[:, ct, :], in_=sview[:, ct, :, :])

            # ---- PE HAM warmup (dummy matmuls during BN; late ones depend on
            # the bn stats so the PE stays busy until the real kq matmuls) ----
            for i in range(6):
                psw = psM.tile([128, 512], F32, tag="m", name="warmmm")
                nc.tensor.matmul(psw[:], wq[i % 4][:, 0:128], wk[i % 4][:, 0:512],
                                 start=True, stop=True)
            statb = stats.bitcast(BF16)
            for i in range(8):
                psw = psM.tile([128, 512], F32, tag="m", name="warmm2")
                nc.tensor.matmul(psw[:], statb[:, 0:128], wk[i % 4][:, 0:512],
                                 start=True, stop=True)

            # ---- batched rstd via rsqrt bit-trick + 2 Newton iterations ----
            r4 = singles.tile([128, 4], F32, name="r4")
            nc.vector.tensor_scalar(out=r4[:], in0=mv[:, :, 1:2], scalar1=EPS,
                                    scalar2=None, op0=mybir.AluOpType.add)
            yi = singles.tile([128, 4], I32, name="yi4")
            with nc.allow_low_precision(reason="rsqrt seed bit trick"):
                nc.vector.tensor_scalar(out=yi[:], in0=r4.bitcast(I32)[:],
                                        scalar1=-1, scalar2=2 * 0x5f3759df,
                                        op0=mybir.AluOpType.mult, op1=mybir.AluOpType.add)
                nc.vector.tensor_scalar(out=yi[:], in0=yi[:], scalar1=1, scalar2=None,
                                        op0=mybir.AluOpType.logical_shift_right)
            rstd = singles.tile([128, 4], F32, name="rstd4")
            yf = yi.bitcast(F32)
            for it in range(2):
                t2 = singles.tile([128, 4], F32, name=f"t2_{it}")
                nc.vector.tensor_tensor(out=t2[:], in0=yf[:], in1=yf[:],
                                        op=mybir.AluOpType.mult)
                nc.vector.tensor_tensor(out=t2[:], in0=t2[:], in1=r4[:],
                                        op=mybir.AluOpType.mult)
                nc.vector.tensor_scalar(out=t2[:], in0=t2[:], scalar1=-0.5,
                                        scalar2=1.5, op0=mybir.AluOpType.mult,
                                        op1=mybir.AluOpType.add)
                dst = rstd if it == 1 else yi.bitcast(F32)
                nc.vector.tensor_tensor(out=dst[:], in0=yf[:], in1=t2[:],
                                        op=mybir.AluOpType.mult)
            s4 = singles.tile([128, 4], F32, name="s4")
            nc.vector.tensor_tensor(out=s4[:], in0=rstd[:], in1=gam[:],
                                    op=mybir.AluOpType.mult)
            t4 = singles.tile([128, 4], F32, name="t4")
            nc.vector.tensor_tensor(out=t4[:], in0=mv[:, :, 0:1], in1=s4[:],
                                    op=mybir.AluOpType.mult)
            nc.vector.tensor_tensor(out=t4[:], in0=bet[:], in1=t4[:],
                                    op=mybir.AluOpType.subtract)
            xn = [singles.tile([128, L], BF16, name=f"xn{i}") for i in range(4)]
            for ct in range(4):
                nc.vector.tensor_scalar(out=xn[ct][:], in0=xch[ct][0][:],
                                        scalar1=s4[:, ct:ct + 1], scalar2=t4[:, ct:ct + 1],
                                        op0=mybir.AluOpType.mult, op1=mybir.AluOpType.add)

            # ---- k/q projections (chunked; bias-add is the PSUM->SBUF move) ----
            kT = [singles.tile([128, L], BF16, name=f"kT{i}") for i in range(4)]
            qT = [singles.tile([128, L], BF16, name=f"qT{i}") for i in range(4)]

            def emit_kq_chunk(mo, which, nh_):
                wmat, bias_t, dst = (wk, bkt, kT) if which == "k" else (wq, bqt, qT)
                ps = psM.tile([128, 512], F32, tag="m", name="ps_kq")
                for kt in range(4):
                    nc.tensor.matmul(ps[:],
                                     wmat[kt][:, mo * 128:(mo + 1) * 128],
                                     xn[kt][:, nh_ * 512:(nh_ + 1) * 512],
                                     start=(kt == 0), stop=(kt == 3))
                nc.vector.tensor_scalar(out=dst[mo][:, nh_ * 512:(nh_ + 1) * 512],
                                        in0=ps[:], scalar1=bias_t[:, mo:mo + 1],
                                        scalar2=None, op0=mybir.AluOpType.add)

            def emit_v(tt):
                ps_full = psM.tile([128, 512], F32, tag="m", name="psv")
                ps = ps_full[:, 0:128]
                for kt in range(4):
                    nc.tensor.matmul(ps[:], xn[kt][:, tt * 128:(tt + 1) * 128],
                                     wv[kt][:], start=(kt == 0), stop=(kt == 3))
                nc.vector.tensor_tensor(
                    out=vsb[:, tt, :, 0:8],
                    in0=ps.rearrange("p (h d) -> p h d", d=8),
                    in1=bvb.rearrange("p (h d) -> p h d", d=8),
                    op=mybir.AluOpType.add)

            KQ_ORDER = [("k", 0), ("q", 0), ("k", 1), ("q", 1)]
            emit_kq_chunk(0, "k", 0)
            emit_kq_chunk(0, "q", 0)
            emit_v(0)
            emit_v(1)

            # ---- attention: 4 quads x (2 h-chunks x 8 f-blocks) ----
            hout = [singles.tile([128, L], BF16, name=f"ho{i}") for i in range(4)]

            def emit_exp(ps, use_act):
                if use_act:
                    wt = wtp.tile([128, 1024], BF16, tag="wt")
                    nc.scalar.activation(out=wt[:], in_=ps[:],
                                         func=mybir.ActivationFunctionType.Exp)
                else:
                    wti = wtp.tile([128, 1024], I16, tag="wt")
                    with nc.allow_low_precision(reason="schraudolph exp approx, validated"):
                        nc.vector.tensor_scalar(
                            out=wti[:], in0=ps[:], scalar1=EXP_A, scalar2=EXP_B,
                            op0=mybir.AluOpType.mult, op1=mybir.AluOpType.add)
                    wt = wti.bitcast(BF16)
                return wt

            rscr_t = rscr_d.tensor if hasattr(rscr_d, "tensor") else rscr_d

            def emit_norm(qd, h, a_sb, rb, mult_dve):
                """Recip the denominators of half h and scale a_sb into hout."""
                hc = slice(h * 512, (h + 1) * 512)
                dt = normp.tile([32, 64], BF16, tag="dt", name="dt")
                for j in range(4):
                    nc.sync.dma_start(
                        dt[8 * j:8 * j + 8, :],
                        a_sb[32 * j + 8:32 * j + 9, hc].rearrange("p (s f) -> p s f", f=64))
                rt = normp.tile([32, 64], BF16, tag="rt", name="rt")
                with nc.allow_low_precision(reason="bf16 softmax denom recip, validated"):
                    nc.vector.reciprocal(out=rt[:], in_=dt[:])
                for j in range(4):
                    hd_ = 4 * qd + j
                    nc.sync.dma_start(
                        rscr_d[hd_, h * 512:(h + 1) * 512].rearrange("(s f) -> s f", f=64),
                        rt[8 * j:8 * j + 8, :])
                for j in range(4):
                    hd_ = 4 * qd + j
                    nc.sync.dma_start(
                        rb[32 * j:32 * j + 32, hc],
                        bass.AP(tensor=rscr_t, offset=hd_ * L + h * 512,
                                ap=[[0, 32], [1, 512]]))
                mul_eng = nc.vector if mult_dve else nc.gpsimd
                mul_eng.tensor_tensor(out=hout[qd][:, hc], in0=a_sb[:, hc], in1=rb[:, hc],
                                      op=mybir.AluOpType.mult)

            def emit_outproj(mo, th, pool, tag):
                ps = pool.tile([128, 512], F32, tag=tag, name="pso")
                for qdd in range(4):
                    nc.tensor.matmul(ps[:], wp[qdd][:, mo * 128:(mo + 1) * 128],
                                     hout[qdd][:, th * 512:(th + 1) * 512],
                                     start=(qdd == 0), stop=(qdd == 3))
                ot = outp.tile([128, 512], F32, tag="ot")
                nc.scalar.activation(out=ot[:], in_=ps[:],
                                     func=mybir.ActivationFunctionType.Copy)
                nc.gpsimd.dma_start(out_d[mo * 128:(mo + 1) * 128, th * 512:(th + 1) * 512],
                                    ot[:])

            for qd in range(4):
                if qd == 1:
                    for ct in range(4):
                        nc.sync.dma_start(wp[ct][:], wp_d[ct * 128:(ct + 1) * 128, :])
                a_sb = normp.tile([128, 1024], BF16, tag="asb")
                rb = normp.tile([128, 1024], BF16, tag="rb")
                av_t = {}
                pend = None  # (wtA, wtB, fb, h)

                for c in range(17):
                    if c < 16:
                        h, fb = divmod(c, 8)
                        fc = slice(fb * 128, (fb + 1) * 128)
                        hc = slice(h * 512, (h + 1) * 512)
                        A = psS.tile([128, 1024], F32, tag="sc", name="ps_scA")
                        B_ = psS.tile([128, 1024], F32, tag="sc", name="ps_scB")
                        nc.tensor.matmul(A[:, 0:512], kT[qd][0:32, fc], qT[qd][0:32, hc],
                                         tile_position=(0, 0))
                        nc.tensor.matmul(A[:, 512:1024], kT[qd][32:64, fc], qT[qd][32:64, hc],
                                         tile_position=(32, 0))
                        nc.tensor.matmul(B_[:, 0:512], kT[qd][64:96, fc], qT[qd][64:96, hc],
                                         tile_position=(64, 0))
                        nc.tensor.matmul(B_[:, 512:1024], kT[qd][96:128, fc], qT[qd][96:128, hc],
                                         tile_position=(96, 0))
                        both_act = c in ACT_BOTH_CYCLES
                        wtA = emit_exp(A, True)
                        wtB = emit_exp(B_, both_act)
                        nxt = (wtA, wtB, fb, h)
                    else:
                        nxt = None

                    if pend is not None:
                        pwtA, pwtB, pfb, ph = pend
                        if pfb == 0:
                            av_t[ph] = psA.tile([128, 512], F32, tag="av", name="av")
                        av = av_t[ph]
                        hd0 = 4 * qd
                        nc.tensor.matmul(av[0:32, :], vsb[:, pfb, hd0 + 0, :], pwtA[:, 0:512],
                                         start=(pfb == 0), stop=(pfb == 7), tile_position=(0, 0))
                        nc.tensor.matmul(av[32:64, :], vsb[:, pfb, hd0 + 1, :], pwtA[:, 512:1024],
                                         start=(pfb == 0), stop=(pfb == 7), tile_position=(0, 32))
                        nc.tensor.matmul(av[64:96, :], vsb[:, pfb, hd0 + 2, :], pwtB[:, 0:512],
                                         start=(pfb == 0), stop=(pfb == 7), tile_position=(0, 64))
                        nc.tensor.matmul(av[96:128, :], vsb[:, pfb, hd0 + 3, :], pwtB[:, 512:1024],
                                         start=(pfb == 0), stop=(pfb == 7), tile_position=(0, 96))
                        if pfb == 7:
                            # dump av(h) -> a_sb chunk (ACT for h0, DVE for h1)
                            dst = a_sb[:, ph * 512:(ph + 1) * 512]
                            if ph == 0:
                                nc.scalar.activation(out=dst, in_=av[:],
                                                     func=mybir.ActivationFunctionType.Copy)
                            else:
                                nc.vector.tensor_copy(dst, av[:])
                            emit_norm(qd, ph, a_sb, rb,
                                      mult_dve=(qd == 3 and ph == 1))
                    pend = nxt

                    # interleaved projections
                    if qd == 0:
                        if c == 0:
                            emit_v(2)
                        elif c == 1:
                            emit_kq_chunk(0, "k", 1)
                            emit_v(3)
                        elif c == 2:
                            emit_kq_chunk(0, "q", 1)
                            emit_v(4)
                        elif c in (3, 4, 5):
                            emit_v(c + 2)
                    if qd < 3 and c in (9, 11, 13, 15):
                        which, nh_ = KQ_ORDER[(c - 9) // 2]
                        emit_kq_chunk(qd + 1, which, nh_)
                    if qd == 3 and c in (13, 15):
                        mo = (c - 13) // 2
                        emit_outproj(mo, 0, psM, "m")

            # ---- output projection remainder (alternate pools) ----
            rem = [(2, 0), (3, 0), (0, 1), (1, 1), (2, 1), (3, 1)]
            for i, (mo, th) in enumerate(rem):
                pool, tag = (psM, "m") if i % 2 == 0 else (psA, "av")
                emit_outproj(mo, th, pool, tag)

    nc.compile()
    return nc


def _prep_inputs(x, gamma, beta, w_qkv, b_qkv, w_proj, b_proj, num_heads):
    """Shard and lay out inputs for the 8 cores."""
    nh = int(num_heads)
    hd = C // nh
    scale = (3 * nh) ** (-0.5)
    wq_full, wk_full, wv_full = w_qkv[0:C], w_qkv[C:2 * C], w_qkv[2 * C:3 * C]
    bq_full, bk_full, bv_full = b_qkv[0:C], b_qkv[C:2 * C], b_qkv[2 * C:3 * C]

    bf16 = ml_dtypes.bfloat16
    in_maps = []
    for core in range(N_CORES):
        bi = core // 4
        g = core % 4
        heads = list(range(HEADS_PER_CORE * g, HEADS_PER_CORE * (g + 1)))
        x_own = x[bi]
        x_oth = x[1 - bi]
        x2 = np.concatenate([x_own, x_oth], axis=1).astype(bf16)

        wqT = np.zeros((C, 512), np.float32)
        wkT = np.zeros((C, 512), np.float32)
        bq = np.zeros(512, np.float32)
        bk = np.zeros(512, np.float32)
        wvT = np.zeros((C, 128), np.float32)
        bv = np.zeros(128, np.float32)
        wpT = np.zeros((512, C), np.float32)
        for jl, h in enumerate(heads):
            for d in range(nh):
                ch = d * hd + h
                wqT[:, 32 * jl + d] = wq_full[ch] * scale
                wkT[:, 32 * jl + d] = wk_full[ch]
                bq[32 * jl + d] = bq_full[ch] * scale
                bk[32 * jl + d] = bk_full[ch]
                wvT[:, 8 * jl + d] = wv_full[ch]
                bv[8 * jl + d] = bv_full[ch]
                wpT[32 * jl + d, :] = w_proj[:, ch]

        in_maps.append({
            "x2": np.ascontiguousarray(x2),
            "gamma": np.ascontiguousarray(gamma.astype(np.float32)),
            "beta": np.ascontiguousarray(beta.astype(np.float32)),
            "wqT": np.ascontiguousarray(wqT.astype(bf16)),
            "wkT": np.ascontiguousarray(wkT.astype(bf16)),
            "wvT": np.ascontiguousarray(wvT.astype(bf16)),
            "bq": bq, "bk": bk, "bv": bv,
            "wpT": np.ascontiguousarray(wpT.astype(bf16)),
        })
    return in_maps


def kernel(x, gamma, beta, w_qkv, b_qkv, w_proj, b_proj, num_heads, _trace=False):
    x = np.asarray(x, dtype=np.float32)
    gamma = np.asarray(gamma, np.float32)
    beta = np.asarray(beta, np.float32)
    w_qkv = np.asarray(w_qkv, np.float32)
    b_qkv = np.asarray(b_qkv, np.float32)
    w_proj = np.asarray(w_proj, np.float32)
    b_proj = np.asarray(b_proj, np.float32)

    if "nc" not in _CACHE:
        _CACHE["nc"] = _build_nc()
    nc = _CACHE["nc"]
    in_maps = _prep_inputs(x, gamma, beta, w_qkv, b_qkv, w_proj, b_proj, num_heads)
    res = run_bass_kernel_spmd(nc, in_maps, core_ids=list(range(N_CORES)), trace=_trace)
    _CACHE["last_result"] = res

    out = np.zeros((B, C, L), np.float32)
    for bi in range(B):
        acc = x[bi] + b_proj[:, None]
        for g in range(4):
            acc = acc + np.asarray(res.results[bi * 4 + g]["out"])
        out[bi] = acc
    return out


# revision 26
# speedup vs baseline: 1.0639x; 1.0274x over previous
"""Trainium2 Bass kernel for nn_AttentionBlock_56075093016781 (8 NeuronCores, SPMD).

Reference semantics (b=2, c=512, L=1024, num_heads=8):
  xn  = batchnorm(x) (stats over batch+length per channel) * gamma + beta
  qkv = w_qkv @ xn + b_qkv                  (1x1 conv over channels)
  layout quirk: qkv -> (b, 3*nh, hd, L) -> (b, hd, L, 3*nh); split q,k,v
    => 64 attention "heads" (the hd axis), feature dim 8 (the nh axis), T=1024
  w   = softmax(scale * q @ k^T) over keys, scale = (3*nh)**-0.5
  a   = w @ v ;  h[d*64+head, t] = a[head, t, d] ;  out = x + w_proj @ h + b_proj

Sharding: 8 cores = 2 batches x 4 head-groups of 16 heads. Each core computes
BN redundantly, its own q/k/v projections, attention for its 16 heads, and a
partial output projection over its 64 channels (padded to 512 rows with zero
weight rows). The host sums the 4 partials per batch and adds the residual
x + b_proj (part of the gather).

Device-side structure (v2 — PE-concurrency rewrite):
  - scores: per (quad, h-chunk of 512 queries, f-block of 128 keys), two
    [128,1024] f32 PSUM tiles each holding 2 heads; the 4 matmuls go to the
    4 distinct 32-row PE tile groups back-to-back so they stream concurrently
  - exp: tile A on ScalarE (ACTIVATE Exp), tile B on VectorE (Schraudolph
    int16 bit-trick -> bitcast bf16); selected cycles send both to ScalarE to
    absorb VectorE's extra (bias/cast/recip) work
  - AV: 4 col-group matmuls (tile_position (0,32j)) accumulate into a single
    1-bank [128,512] PSUM accumulator per (quad, h); softmax denominator
    comes free from a ones-column in the v blocks
  - PSUM budget: 3x2 banks (scores) + 1 (av) + 1 (kq/v/outproj misc) = 8
  - kq projections for quad qd+1 and the v projection are interleaved into
    the attention cycles of quad qd / quad 0
  - ACT exp-table preload + PE HAM-warmup dummy matmuls run during BN
  - BN rstd via rsqrt bit-trick + 2 Newton steps, batched over all 4
    channel blocks; x shipped as bf16
"""
import numpy as np
import ml_dtypes

import concourse.bass as bass
import concourse.bacc as bacc
import concourse.mybir as mybir
import concourse.tile as tile
from concourse.bass_utils import run_bass_kernel_spmd

F32 = mybir.dt.float32
BF16 = mybir.dt.bfloat16
I16 = mybir.dt.int16
I32 = mybir.dt.int32

B, C, L = 2, 512, 1024
NH = 8          # feature dim of each attention head (from num_heads)
HD = 64         # number of attention heads (head_dim axis of the quirky layout)
HEADS_PER_CORE = 16
N_CORES = 8
EPS = 1e-5

# exp-assignment: cycles (of 16 per quad) where BOTH score tiles go to ScalarE
ACT_BOTH_CYCLES = (5, 13)

_CACHE = {}


def _build_nc():
    nc = bacc.Bacc(None, target_bir_lowering=False)

    # ---- DRAM I/O ----
    x2_d = nc.dram_tensor("x2", [C, 2 * L], BF16, kind="ExternalInput")      # [c, b*L]
    gamma_d = nc.dram_tensor("gamma", [C], F32, kind="ExternalInput")
    beta_d = nc.dram_tensor("beta", [C], F32, kind="ExternalInput")
    wq_d = nc.dram_tensor("wqT", [C, 512], BF16, kind="ExternalInput")       # [c, padded qch] (scale folded)
    wk_d = nc.dram_tensor("wkT", [C, 512], BF16, kind="ExternalInput")
    wv_d = nc.dram_tensor("wvT", [C, 128], BF16, kind="ExternalInput")       # [c, vch compact]
    bq_d = nc.dram_tensor("bq", [512], F32, kind="ExternalInput")            # padded, scale folded
    bk_d = nc.dram_tensor("bk", [512], F32, kind="ExternalInput")
    bv_d = nc.dram_tensor("bv", [128], F32, kind="ExternalInput")
    wp_d = nc.dram_tensor("wpT", [512, 512], BF16, kind="ExternalInput")     # [padded c, o]
    out_d = nc.dram_tensor("out", [C, L], F32, kind="ExternalOutput")
    rscr_d = nc.dram_tensor("rscr", [HEADS_PER_CORE, L], BF16)               # internal scratch (recip denoms)

    EXP_A = 184.66496
    EXP_B = 16248.75

    with tile.TileContext(nc) as tc:
        with (
            tc.tile_pool(name="singles", bufs=1) as singles,
            tc.tile_pool(name="wt", bufs=6) as wtp,
            tc.tile_pool(name="norm", bufs=3) as normp,
            tc.tile_pool(name="outp", bufs=3) as outp,
            tc.tile_pool(name="psS", bufs=3, space="PSUM") as psS,
            tc.tile_pool(name="psA", bufs=1, space="PSUM") as psA,
            tc.tile_pool(name="psM", bufs=1, space="PSUM") as psM,
        ):
            # ---- input DMAs (x2 chunks first on every queue; weights after) ----
            wq = [singles.tile([128, 512], BF16, name=f"wq{i}") for i in range(4)]
            wk = [singles.tile([128, 512], BF16, name=f"wk{i}") for i in range(4)]
            wv = [singles.tile([128, 128], BF16, name=f"wv{i}") for i in range(4)]
            wp = [singles.tile([128, 512], BF16, name=f"wp{i}") for i in range(4)]
            xch = [[singles.tile([128, 1024], BF16, name=f"xc{i}_{k}") for k in range(2)]
                   for i in range(4)]
            for ct in range(4):
                for k in range(2):
                    (nc.sync if k == 0 else nc.scalar).dma_start(
                        xch[ct][k][:], x2_d[ct * 128:(ct + 1) * 128, k * 1024:(k + 1) * 1024])
            for ct in range(4):
                nc.gpsimd.dma_start(wq[ct][:], wq_d[ct * 128:(ct + 1) * 128, :])
                nc.gpsimd.dma_start(wk[ct][:], wk_d[ct * 128:(ct + 1) * 128, :])
                nc.gpsimd.dma_start(wv[ct][:], wv_d[ct * 128:(ct + 1) * 128, :])
            gam = singles.tile([128, 4], F32, name="gam")
            bet = singles.tile([128, 4], F32, name="bet")
            nc.gpsimd.dma_start(gam[:], gamma_d.rearrange("(o p) -> p o", p=128))
            nc.gpsimd.dma_start(bet[:], beta_d.rearrange("(o p) -> p o", p=128))
            bqt = singles.tile([128, 4], F32, name="bqt")
            bkt = singles.tile([128, 4], F32, name="bkt")
            nc.gpsimd.dma_start(bqt[:], bq_d.rearrange("(o p) -> p o", p=128))
            nc.gpsimd.dma_start(bkt[:], bk_d.rearrange("(o p) -> p o", p=128))
            bvb = singles.tile([128, 128], F32, name="bvb")  # bv broadcast across partitions
            nc.gpsimd.dma_start(bvb[:], bass.AP(tensor=bv_d.tensor if hasattr(bv_d, "tensor") else bv_d,
                                                offset=0, ap=[[0, 128], [1, 128]]))

            # ---- ACT exp-table preload + vsb init (runs while DMA/BN proceed) ----
            epst = singles.tile([128, 1], F32, name="eps")
            nc.vector.memset(epst[:], EPS)
            warm = singles.tile([128, 1], BF16, name="warm")
            nc.scalar.activation(out=warm[:], in_=epst[:],
                                 func=mybir.ActivationFunctionType.Exp)

            # vsb[f, fb, head, 0:8]=v, [..,8]=1, [..,9:32]=0
            vsb = singles.tile([128, 8, 16, 32], BF16, name="vsb")
            nc.vector.memset(vsb[:], 0.0)
            nc.vector.memset(vsb[:, :, :, 8:9], 1.0)

            # ---- BatchNorm stats (over both batches) ----
            stats = singles.tile([128, 96], F32, name="bnstats")
            sview = stats.rearrange("p (c s d) -> p c s d", c=4, s=4)
            for ct in range(4):
                for k in range(2):
                    xin = xch[ct][k].rearrange("p (s f) -> p s f", f=512)
                    for si in range(2):
                        nc.vector.bn_stats(out=sview[:, ct, 2 * k + si, :], in_=xin[:, si, :])
            mv = singles.tile([128, 4, 2], F32, name="mv")
            for ct in range(4):
                nc.vector.bn_aggr(out=mv[:, ct, :], in_=sview[:, ct, :, :])

            # ---- PE HAM warmup (dummy matmuls during BN; late ones depend on
            # the bn stats so the PE stays busy until the real kq matmuls) ----
            for i in range(6):
                psw = psM.tile([128, 512], F32, tag="m", name="warmmm")
                nc.tensor.matmul(psw[:], xch[0][0][:, 0:128], xch[0][0][:, 0:512],
                                 start=True, stop=True)
            statb = stats.bitcast(BF16)
            for i in range(8):
                psw = psM.tile([128, 512], F32, tag="m", name="warmm2")
                nc.tensor.matmul(psw[:], statb[:, 0:128], xch[1][0][:, 0:512],
                                 start=True, stop=True)

            # ---- batched rstd via rsqrt bit-trick + 2 Newton iterations ----
            r4 = singles.tile([128, 4], F32, name="r4")
            nc.vector.tensor_scalar(out=r4[:], in0=mv[:, :, 1:2], scalar1=EPS,
                                    scalar2=None, op0=mybir.AluOpType.add)
            yi = singles.tile([128, 4], I32, name="yi4")
            with nc.allow_low_precision(reason="rsqrt seed bit trick"):
                nc.vector.tensor_scalar(out=yi[:], in0=r4.bitcast(I32)[:],
                                        scalar1=-1, scalar2=2 * 0x5f3759df,
                                        op0=mybir.AluOpType.mult, op1=mybir.AluOpType.add)
                nc.vector.tensor_scalar(out=yi[:], in0=yi[:], scalar1=1, scalar2=None,
                                        op0=mybir.AluOpType.logical_shift_right)
            rstd = singles.tile([128, 4], F32, name="rstd4")
            yf = yi.bitcast(F32)
            for it in range(2):
                t2 = singles.tile([128, 4], F32, name=f"t2_{it}")
                nc.vector.tensor_tensor(out=t2[:], in0=yf[:], in1=yf[:],
                                        op=mybir.AluOpType.mult)
                nc.vector.tensor_tensor(out=t2[:], in0=t2[:], in1=r4[:],
                                        op=mybir.AluOpType.mult)
                nc.vector.tensor_scalar(out=t2[:], in0=t2[:], scalar1=-0.5,
                                        scalar2=1.5, op0=mybir.AluOpType.mult,
                                        op1=mybir.AluOpType.add)
                dst = rstd if it == 1 else yi.bitcast(F32)
                nc.vector.tensor_tensor(out=dst[:], in0=yf[:], in1=t2[:],
                                        op=mybir.AluOpType.mult)
            s4 = singles.tile([128, 4], F32, name="s4")
            nc.vector.tensor_tensor(out=s4[:], in0=rstd[:], in1=gam[:],
                                    op=mybir.AluOpType.mult)
            t4 = singles.tile([128, 4], F32, name="t4")
            nc.vector.tensor_tensor(out=t4[:], in0=mv[:, :, 0:1], in1=s4[:],
                                    op=mybir.AluOpType.mult)
            nc.vector.tensor_tensor(out=t4[:], in0=bet[:], in1=t4[:],
                                    op=mybir.AluOpType.subtract)
            xn = [singles.tile([128, L], BF16, name=f"xn{i}") for i in range(4)]
            for ct in range(4):
                nc.vector.tensor_scalar(out=xn[ct][:], in0=xch[ct][0][:],
                                        scalar1=s4[:, ct:ct + 1], scalar2=t4[:, ct:ct + 1],
                                        op0=mybir.AluOpType.mult, op1=mybir.AluOpType.add)

            # ---- k/q projections (chunked; bias-add is the PSUM->SBUF move) ----
            kT = [singles.tile([128, L], BF16, name=f"kT{i}") for i in range(4)]
            qT = [singles.tile([128, L], BF16, name=f"qT{i}") for i in range(4)]

            def emit_kq_chunk(mo, which, nh_):
                wmat, bias_t, dst = (wk, bkt, kT) if which == "k" else (wq, bqt, qT)
                ps = psM.tile([128, 512], F32, tag="m", name="ps_kq")
                for kt in range(4):
                    nc.tensor.matmul(ps[:],
                                     wmat[kt][:, mo * 128:(mo + 1) * 128],
                                     xn[kt][:, nh_ * 512:(nh_ + 1) * 512],
                                     start=(kt == 0), stop=(kt == 3))
                nc.vector.tensor_scalar(out=dst[mo][:, nh_ * 512:(nh_ + 1) * 512],
                                        in0=ps[:], scalar1=bias_t[:, mo:mo + 1],
                                        scalar2=None, op0=mybir.AluOpType.add)

            def emit_v(tt):
                ps_full = psM.tile([128, 512], F32, tag="m", name="psv")
                ps = ps_full[:, 0:128]
                for kt in range(4):
                    nc.tensor.matmul(ps[:], xn[kt][:, tt * 128:(tt + 1) * 128],
                                     wv[kt][:], start=(kt == 0), stop=(kt == 3))
                nc.vector.tensor_tensor(
                    out=vsb[:, tt, :, 0:8],
                    in0=ps.rearrange("p (h d) -> p h d", d=8),
                    in1=bvb.rearrange("p (h d) -> p h d", d=8),
                    op=mybir.AluOpType.add)

            KQ_ORDER = [("k", 0), ("q", 0), ("k", 1), ("q", 1)]
            emit_kq_chunk(0, "k", 0)
            emit_kq_chunk(0, "q", 0)
            emit_v(0)
            emit_v(1)

            # ---- attention: 4 quads x (2 h-chunks x 8 f-blocks) ----
            hout = [singles.tile([128, L], BF16, name=f"ho{i}") for i in range(4)]

            def emit_exp(ps, use_act):
                if use_act:
                    wt = wtp.tile([128, 1024], BF16, tag="wt")
                    nc.scalar.activation(out=wt[:], in_=ps[:],
                                         func=mybir.ActivationFunctionType.Exp)
                else:
                    wti = wtp.tile([128, 1024], I16, tag="wt")
                    with nc.allow_low_precision(reason="schraudolph exp approx, validated"):
                        nc.vector.tensor_scalar(
                            out=wti[:], in0=ps[:], scalar1=EXP_A, scalar2=EXP_B,
                            op0=mybir.AluOpType.mult, op1=mybir.AluOpType.add)
                    wt = wti.bitcast(BF16)
                return wt

            rscr_t = rscr_d.tensor if hasattr(rscr_d, "tensor") else rscr_d

            def emit_norm(qd, h, a_sb, rb, mult_dve):
                """Recip the denominators of half h and scale a_sb into hout."""
                hc = slice(h * 512, (h + 1) * 512)
                dt = normp.tile([32, 64], BF16, tag="dt", name="dt")
                for j in range(4):
                    nc.sync.dma_start(
                        dt[8 * j:8 * j + 8, :],
                        a_sb[32 * j + 8:32 * j + 9, hc].rearrange("p (s f) -> p s f", f=64))
                rt = normp.tile([32, 64], BF16, tag="rt", name="rt")
                with nc.allow_low_precision(reason="bf16 softmax denom recip, validated"):
                    nc.vector.reciprocal(out=rt[:], in_=dt[:])
                for j in range(4):
                    hd_ = 4 * qd + j
                    nc.sync.dma_start(
                        rscr_d[hd_, h * 512:(h + 1) * 512].rearrange("(s f) -> s f", f=64),
                        rt[8 * j:8 * j + 8, :])
                for j in range(4):
                    hd_ = 4 * qd + j
                    nc.sync.dma_start(
                        rb[32 * j:32 * j + 32, hc],
                        bass.AP(tensor=rscr_t, offset=hd_ * L + h * 512,
                                ap=[[0, 32], [1, 512]]))
                mul_eng = nc.vector if mult_dve else nc.gpsimd
                mul_eng.tensor_tensor(out=hout[qd][:, hc], in0=a_sb[:, hc], in1=rb[:, hc],
                                      op=mybir.AluOpType.mult)

            def emit_outproj(mo, th, pool, tag):
                if tag == "sc":
                    ps_full = pool.tile([128, 1024], F32, tag=tag, name="pso")
                    ps = ps_full[:, 0:512]
                else:
                    ps = pool.tile([128, 512], F32, tag=tag, name="pso")
                for qdd in range(4):
                    nc.tensor.matmul(ps[:], wp[qdd][:, mo * 128:(mo + 1) * 128],
                                     hout[qdd][:, th * 512:(th + 1) * 512],
                                     start=(qdd == 0), stop=(qdd == 3))
                ot = outp.tile([128, 512], F32, tag="ot")
                nc.scalar.activation(out=ot[:], in_=ps[:],
                                     func=mybir.ActivationFunctionType.Copy)
                nc.gpsimd.dma_start(out_d[mo * 128:(mo + 1) * 128, th * 512:(th + 1) * 512],
                                    ot[:])

            av_t = {}
            pends = []  # fifo of dicts, AV runs 2 cycles behind its scores

            def process_pend(p):
                pqd, pfb, ph = p["qd"], p["fb"], p["h"]
                if pfb == 0:
                    av_t[ph] = psA.tile([128, 512], F32, tag="av", name="av")
                av = av_t[ph]
                hd0 = 4 * pqd
                pwtA, pwtB = p["wtA"], p["wtB"]
                nc.tensor.matmul(av[0:32, :], vsb[:, pfb, hd0 + 0, :], pwtA[:, 0:512],
                                 start=(pfb == 0), stop=(pfb == 7), tile_position=(0, 0))
                nc.tensor.matmul(av[32:64, :], vsb[:, pfb, hd0 + 1, :], pwtA[:, 512:1024],
                                 start=(pfb == 0), stop=(pfb == 7), tile_position=(0, 32))
                nc.tensor.matmul(av[64:96, :], vsb[:, pfb, hd0 + 2, :], pwtB[:, 0:512],
                                 start=(pfb == 0), stop=(pfb == 7), tile_position=(0, 64))
                nc.tensor.matmul(av[96:128, :], vsb[:, pfb, hd0 + 3, :], pwtB[:, 512:1024],
                                 start=(pfb == 0), stop=(pfb == 7), tile_position=(0, 96))
                if pfb == 7:
                    # dump av(h) -> a_sb chunk (ACT for h0, DVE for h1); this
                    # is emitted before this cycle's exps so it lands early in
                    # the engine queue
                    dst = p["a_sb"][:, ph * 512:(ph + 1) * 512]
                    if ph == 0:
                        nc.scalar.activation(out=dst, in_=av[:],
                                             func=mybir.ActivationFunctionType.Copy)
                    else:
                        nc.vector.tensor_copy(dst, av[:])
                    emit_norm(pqd, ph, p["a_sb"], p["rb"],
                              mult_dve=(pqd == 3 and ph == 1))

            for qd in range(4):
                if qd == 1:
                    for ct in range(4):
                        nc.gpsimd.dma_start(wp[ct][:], wp_d[ct * 128:(ct + 1) * 128, :])
                a_sb = normp.tile([128, 1024], BF16, tag="asb")
                rb = normp.tile([128, 1024], BF16, tag="rb")

                for c in range(16):
                    h, fb = divmod(c, 8)
                    fc = slice(fb * 128, (fb + 1) * 128)
                    hc = slice(h * 512, (h + 1) * 512)
                    A = psS.tile([128, 1024], F32, tag="sc", name="ps_scA")
                    B_ = psS.tile([128, 1024], F32, tag="sc", name="ps_scB")
                    nc.tensor.matmul(A[:, 0:512], kT[qd][0:32, fc], qT[qd][0:32, hc],
                                     tile_position=(0, 0))
                    nc.tensor.matmul(A[:, 512:1024], kT[qd][32:64, fc], qT[qd][32:64, hc],
                                     tile_position=(32, 0))
                    nc.tensor.matmul(B_[:, 0:512], kT[qd][64:96, fc], qT[qd][64:96, hc],
                                     tile_position=(64, 0))
                    nc.tensor.matmul(B_[:, 512:1024], kT[qd][96:128, fc], qT[qd][96:128, hc],
                                     tile_position=(96, 0))

                    if len(pends) >= 2:
                        process_pend(pends.pop(0))

                    both_act = c in ACT_BOTH_CYCLES
                    wtA = emit_exp(A, True)
                    wtB = emit_exp(B_, both_act)
                    pends.append({"wtA": wtA, "wtB": wtB, "fb": fb, "h": h,
                                  "qd": qd, "a_sb": a_sb, "rb": rb})

                    # interleaved projections
                    if qd == 0:
                        if c == 0:
                            emit_v(2)
                        elif c == 1:
                            emit_kq_chunk(0, "k", 1)
                            emit_v(3)
                        elif c == 2:
                            emit_kq_chunk(0, "q", 1)
                            emit_v(4)
                        elif c in (3, 4, 5):
                            emit_v(c + 2)
                    if qd < 3 and c in (9, 11, 13, 15):
                        which, nh_ = KQ_ORDER[(c - 9) // 2]
                        emit_kq_chunk(qd + 1, which, nh_)
                    if qd == 3 and c in (13, 15):
                        mo = (c - 13) // 2
                        emit_outproj(mo, 0, psM, "m")
            while pends:
                process_pend(pends.pop(0))

            # ---- output projection remainder (rotate pools for pipelining) ----
            rem = [(2, 0), (3, 0), (0, 1), (1, 1), (2, 1), (3, 1)]
            pools = [(psS, "sc"), (psM, "m"), (psA, "av")]
            for i, (mo, th) in enumerate(rem):
                pool, tag = pools[i % 3]
                emit_outproj(mo, th, pool, tag)

    nc.compile()
    return nc


def _prep_inputs(x, gamma, beta, w_qkv, b_qkv, w_proj, b_proj, num_heads):
    """Shard and lay out inputs for the 8 cores."""
    nh = int(num_heads)
    hd = C // nh
    scale = (3 * nh) ** (-0.5)
    wq_full, wk_full, wv_full = w_qkv[0:C], w_qkv[C:2 * C], w_qkv[2 * C:3 * C]
    bq_full, bk_full, bv_full = b_qkv[0:C], b_qkv[C:2 * C], b_qkv[2 * C:3 * C]

    bf16 = ml_dtypes.bfloat16
    in_maps = []
    for core in range(N_CORES):
        bi = core // 4
        g = core % 4
        heads = list(range(HEADS_PER_CORE * g, HEADS_PER_CORE * (g + 1)))
        x_own = x[bi]
        x_oth = x[1 - bi]
        x2 = np.concatenate([x_own, x_oth], axis=1).astype(bf16)

        wqT = np.zeros((C, 512), np.float32)
        wkT = np.zeros((C, 512), np.float32)
        bq = np.zeros(512, np.float32)
        bk = np.zeros(512, np.float32)
        wvT = np.zeros((C, 128), np.float32)
        bv = np.zeros(128, np.float32)
        wpT = np.zeros((512, C), np.float32)
        for jl, h in enumerate(heads):
            for d in range(nh):
                ch = d * hd + h
                wqT[:, 32 * jl + d] = wq_full[ch] * scale
                wkT[:, 32 * jl + d] = wk_full[ch]
                bq[32 * jl + d] = bq_full[ch] * scale
                bk[32 * jl + d] = bk_full[ch]
                wvT[:, 8 * jl + d] = wv_full[ch]
                bv[8 * jl + d] = bv_full[ch]
                wpT[32 * jl + d, :] = w_proj[:, ch]

        in_maps.append({
            "x2": np.ascontiguousarray(x2),
            "gamma": np.ascontiguousarray(gamma.astype(np.float32)),
            "beta": np.ascontiguousarray(beta.astype(np.float32)),
            "wqT": np.ascontiguousarray(wqT.astype(bf16)),
            "wkT": np.ascontiguousarray(wkT.astype(bf16)),
            "wvT": np.ascontiguousarray(wvT.astype(bf16)),
            "bq": bq, "bk": bk, "bv": bv,
            "wpT": np.ascontiguousarray(wpT.astype(bf16)),
        })
    return in_maps


def kernel(x, gamma, beta, w_qkv, b_qkv, w_proj, b_proj, num_heads, _trace=False):
    x = np.asarray(x, dtype=np.float32)
    gamma = np.asarray(gamma, np.float32)
    beta = np.asarray(beta, np.float32)
    w_qkv = np.asarray(w_qkv, np.float32)
    b_qkv = np.asarray(b_qkv, np.float32)
    w_proj = np.asarray(w_proj, np.float32)
    b_proj = np.asarray(b_proj, np.float32)

    if "nc" not in _CACHE:
        _CACHE["nc"] = _build_nc()
    nc = _CACHE["nc"]
    in_maps = _prep_inputs(x, gamma, beta, w_qkv, b_qkv, w_proj, b_proj, num_heads)
    res = run_bass_kernel_spmd(nc, in_maps, core_ids=list(range(N_CORES)), trace=_trace)
    _CACHE["last_result"] = res

    out = np.zeros((B, C, L), np.float32)
    for bi in range(B):
        acc = x[bi] + b_proj[:, None]
        for g in range(4):
            acc = acc + np.asarray(res.results[bi * 4 + g]["out"])
        out[bi] = acc
    return out


# revision 29
# speedup vs baseline: 1.1133x; 1.0464x over previous
"""Trainium2 Bass kernel for nn_AttentionBlock_56075093016781 (8 NeuronCores, SPMD).

Reference semantics (b=2, c=512, L=1024, num_heads=8):
  xn  = batchnorm(x) (stats over batch+length per channel) * gamma + beta
  qkv = w_qkv @ xn + b_qkv                  (1x1 conv over channels)
  layout quirk: qkv -> (b, 3*nh, hd, L) -> (b, hd, L, 3*nh); split q,k,v
    => 64 attention "heads" (the hd axis), feature dim 8 (the nh axis), T=1024
  w   = softmax(scale * q @ k^T) over keys, scale = (3*nh)**-0.5
  a   = w @ v ;  h[d*64+head, t] = a[head, t, d] ;  out = x + w_proj @ h + b_proj

Sharding: 8 cores = 2 batches x 4 head-groups of 16 heads. Each core computes
BN redundantly, its own q/k/v projections, attention for its 16 heads, and a
partial output projection over its 64 channels (padded to 512 rows with zero
weight rows). The host sums the 4 partials per batch and adds the residual
x + b_proj (part of the gather).

Device-side structure (v2 — PE-concurrency rewrite):
  - scores: per (quad, h-chunk of 512 queries, f-block of 128 keys), two
    [128,1024] f32 PSUM tiles each holding 2 heads; the 4 matmuls go to the
    4 distinct 32-row PE tile groups back-to-back so they stream concurrently
  - exp: tile A on ScalarE (ACTIVATE Exp), tile B on VectorE (Schraudolph
    int16 bit-trick -> bitcast bf16); selected cycles send both to ScalarE to
    absorb VectorE's extra (bias/cast/recip) work
  - AV: 4 col-group matmuls (tile_position (0,32j)) accumulate into a single
    1-bank [128,512] PSUM accumulator per (quad, h); softmax denominator
    comes free from a ones-column in the v blocks
  - PSUM budget: 3x2 banks (scores) + 1 (av) + 1 (kq/v/outproj misc) = 8
  - kq projections for quad qd+1 and the v projection are interleaved into
    the attention cycles of quad qd / quad 0
  - ACT exp-table preload + PE HAM-warmup dummy matmuls run during BN
  - BN rstd via rsqrt bit-trick + 2 Newton steps, batched over all 4
    channel blocks; x shipped as bf16
"""
import numpy as np
import ml_dtypes

import concourse.bass as bass
import concourse.bacc as bacc
import concourse.mybir as mybir
import concourse.tile as tile
from concourse.bass_utils import run_bass_kernel_spmd

F32 = mybir.dt.float32
BF16 = mybir.dt.bfloat16
I16 = mybir.dt.int16
I32 = mybir.dt.int32

B, C, L = 2, 512, 1024
NH = 8          # feature dim of each attention head (from num_heads)
HD = 64         # number of attention heads (head_dim axis of the quirky layout)
HEADS_PER_CORE = 16
N_CORES = 8
EPS = 1e-5

# exp-assignment: cycles (of 16 per quad) where BOTH score tiles go to ScalarE
ACT_BOTH_CYCLES = (5, 13)

_CACHE = {}


def _build_nc():
    nc = bacc.Bacc(None, target_bir_lowering=False)

    # ---- DRAM I/O ----
    x2_d = nc.dram_tensor("x2", [C, 2 * L], BF16, kind="ExternalInput")      # [c, b*L]
    gamma_d = nc.dram_tensor("gamma", [C], F32, kind="ExternalInput")
    beta_d = nc.dram_tensor("beta", [C], F32, kind="ExternalInput")
    wq_d = nc.dram_tensor("wqT", [C, 512], BF16, kind="ExternalInput")       # [c, padded qch] (scale folded)
    wk_d = nc.dram_tensor("wkT", [C, 512], BF16, kind="ExternalInput")
    wv_d = nc.dram_tensor("wvT", [C, 128], BF16, kind="ExternalInput")       # [c, vch compact]
    bq_d = nc.dram_tensor("bq", [512], F32, kind="ExternalInput")            # padded, scale folded
    bk_d = nc.dram_tensor("bk", [512], F32, kind="ExternalInput")
    bv_d = nc.dram_tensor("bv", [128], F32, kind="ExternalInput")
    wp_d = nc.dram_tensor("wpT", [512, 512], BF16, kind="ExternalInput")     # [padded c, o]
    out_d = nc.dram_tensor("out", [C, L], F32, kind="ExternalOutput")
    rscr_d = nc.dram_tensor("rscr", [HEADS_PER_CORE, L], BF16)               # internal scratch (recip denoms)

    EXP_A = 184.66496
    EXP_B = 16248.75

    with tile.TileContext(nc) as tc:
        with (
            tc.tile_pool(name="singles", bufs=1) as singles,
            tc.tile_pool(name="wt", bufs=6) as wtp,
            tc.tile_pool(name="norm", bufs=3) as normp,
            tc.tile_pool(name="outp", bufs=3) as outp,
            tc.tile_pool(name="psS", bufs=3, space="PSUM") as psS,
            tc.tile_pool(name="psA", bufs=1, space="PSUM") as psA,
            tc.tile_pool(name="psM", bufs=1, space="PSUM") as psM,
        ):
            # ---- input DMAs (x2 chunks first on every queue; weights after) ----
            wq = [singles.tile([128, 512], BF16, name=f"wq{i}") for i in range(4)]
            wk = [singles.tile([128, 512], BF16, name=f"wk{i}") for i in range(4)]
            wv = [singles.tile([128, 128], BF16, name=f"wv{i}") for i in range(4)]
            wp = [singles.tile([128, 512], BF16, name=f"wp{i}") for i in range(4)]
            xch = [[singles.tile([128, 1024], BF16, name=f"xc{i}_{k}") for k in range(2)]
                   for i in range(4)]
            for ct in range(4):
                for k in range(2):
                    (nc.sync if k == 0 else nc.scalar).dma_start(
                        xch[ct][k][:], x2_d[ct * 128:(ct + 1) * 128, k * 1024:(k + 1) * 1024])
            for ct in range(4):
                nc.scalar.dma_start(wq[ct][:], wq_d[ct * 128:(ct + 1) * 128, :])
                nc.sync.dma_start(wk[ct][:], wk_d[ct * 128:(ct + 1) * 128, :])
                nc.gpsimd.dma_start(wv[ct][:], wv_d[ct * 128:(ct + 1) * 128, :])
            gam = singles.tile([128, 4], F32, name="gam")
            bet = singles.tile([128, 4], F32, name="bet")
            nc.gpsimd.dma_start(gam[:], gamma_d.rearrange("(o p) -> p o", p=128))
            nc.gpsimd.dma_start(bet[:], beta_d.rearrange("(o p) -> p o", p=128))
            bqt = singles.tile([128, 4], F32, name="bqt")
            bkt = singles.tile([128, 4], F32, name="bkt")
            nc.gpsimd.dma_start(bqt[:], bq_d.rearrange("(o p) -> p o", p=128))
            nc.gpsimd.dma_start(bkt[:], bk_d.rearrange("(o p) -> p o", p=128))
            bvb = singles.tile([128, 128], F32, name="bvb")  # bv broadcast across partitions
            nc.gpsimd.dma_start(bvb[:], bass.AP(tensor=bv_d.tensor if hasattr(bv_d, "tensor") else bv_d,
                                                offset=0, ap=[[0, 128], [1, 128]]))

            # ---- ACT exp-table preload + vsb init (runs while DMA/BN proceed) ----
            epst = singles.tile([128, 1], F32, name="eps")
            nc.vector.memset(epst[:], EPS)
            warm = singles.tile([128, 1], BF16, name="warm")
            nc.scalar.activation(out=warm[:], in_=epst[:],
                                 func=mybir.ActivationFunctionType.Exp)

            # vsb[f, fb, head, 0:8]=v, [..,8]=1, [..,9:32]=0
            vsb = singles.tile([128, 8, 16, 32], BF16, name="vsb")
            nc.vector.memset(vsb[:], 0.0)
            nc.vector.memset(vsb[:, :, :, 8:9], 1.0)

            # ---- BatchNorm stats (over both batches) ----
            stats = singles.tile([128, 96], F32, name="bnstats")
            sview = stats.rearrange("p (c s d) -> p c s d", c=4, s=4)
            for ct in range(4):
                for k in range(2):
                    xin = xch[ct][k].rearrange("p (s f) -> p s f", f=512)
                    for si in range(2):
                        nc.vector.bn_stats(out=sview[:, ct, 2 * k + si, :], in_=xin[:, si, :])
            mv = singles.tile([128, 4, 2], F32, name="mv")
            for ct in range(4):
                nc.vector.bn_aggr(out=mv[:, ct, :], in_=sview[:, ct, :, :])

            # ---- PE HAM warmup (dummy matmuls gated on successive x2 chunks
            # so the PE stays busy from the first chunk until the kq matmuls) ----
            for src in (xch[0][0], xch[1][1], xch[2][1], xch[3][1]):
                for i in range(4):
                    psw = psM.tile([128, 512], F32, tag="m", name="warmmm")
                    nc.tensor.matmul(psw[:], src[:, 0:128], src[:, 0:512],
                                     start=True, stop=True)

            # ---- batched rstd via rsqrt bit-trick + 2 Newton iterations ----
            r4 = singles.tile([128, 4], F32, name="r4")
            nc.vector.tensor_scalar(out=r4[:], in0=mv[:, :, 1:2], scalar1=EPS,
                                    scalar2=None, op0=mybir.AluOpType.add)
            yi = singles.tile([128, 4], I32, name="yi4")
            with nc.allow_low_precision(reason="rsqrt seed bit trick"):
                nc.vector.tensor_scalar(out=yi[:], in0=r4.bitcast(I32)[:],
                                        scalar1=-1, scalar2=2 * 0x5f3759df,
                                        op0=mybir.AluOpType.mult, op1=mybir.AluOpType.add)
                nc.vector.tensor_scalar(out=yi[:], in0=yi[:], scalar1=1, scalar2=None,
                                        op0=mybir.AluOpType.logical_shift_right)
            rstd = singles.tile([128, 4], F32, name="rstd4")
            yf = yi.bitcast(F32)
            for it in range(2):
                t2 = singles.tile([128, 4], F32, name=f"t2_{it}")
                nc.vector.tensor_tensor(out=t2[:], in0=yf[:], in1=yf[:],
                                        op=mybir.AluOpType.mult)
                nc.vector.tensor_tensor(out=t2[:], in0=t2[:], in1=r4[:],
                                        op=mybir.AluOpType.mult)
                nc.vector.tensor_scalar(out=t2[:], in0=t2[:], scalar1=-0.5,
                                        scalar2=1.5, op0=mybir.AluOpType.mult,
                                        op1=mybir.AluOpType.add)
                dst = rstd if it == 1 else yi.bitcast(F32)
                nc.vector.tensor_tensor(out=dst[:], in0=yf[:], in1=t2[:],
                                        op=mybir.AluOpType.mult)
            s4 = singles.tile([128, 4], F32, name="s4")
            nc.vector.tensor_tensor(out=s4[:], in0=rstd[:], in1=gam[:],
                                    op=mybir.AluOpType.mult)
            t4 = singles.tile([128, 4], F32, name="t4")
            nc.vector.tensor_tensor(out=t4[:], in0=mv[:, :, 0:1], in1=s4[:],
                                    op=mybir.AluOpType.mult)
            nc.vector.tensor_tensor(out=t4[:], in0=bet[:], in1=t4[:],
                                    op=mybir.AluOpType.subtract)
            xn = [singles.tile([128, L], BF16, name=f"xn{i}") for i in range(4)]
            for ct in range(4):
                nc.vector.tensor_scalar(out=xn[ct][:], in0=xch[ct][0][:],
                                        scalar1=s4[:, ct:ct + 1], scalar2=t4[:, ct:ct + 1],
                                        op0=mybir.AluOpType.mult, op1=mybir.AluOpType.add)

            # ---- k/q projections (chunked; bias-add is the PSUM->SBUF move) ----
            kT = [singles.tile([128, L], BF16, name=f"kT{i}") for i in range(4)]
            qT = [singles.tile([128, L], BF16, name=f"qT{i}") for i in range(4)]

            def emit_kq_chunk(mo, which, nh_):
                wmat, bias_t, dst = (wk, bkt, kT) if which == "k" else (wq, bqt, qT)
                ps = psM.tile([128, 512], F32, tag="m", name="ps_kq")
                for kt in range(4):
                    nc.tensor.matmul(ps[:],
                                     wmat[kt][:, mo * 128:(mo + 1) * 128],
                                     xn[kt][:, nh_ * 512:(nh_ + 1) * 512],
                                     start=(kt == 0), stop=(kt == 3))
                nc.vector.tensor_scalar(out=dst[mo][:, nh_ * 512:(nh_ + 1) * 512],
                                        in0=ps[:], scalar1=bias_t[:, mo:mo + 1],
                                        scalar2=None, op0=mybir.AluOpType.add)

            def emit_v(tt):
                ps_full = psM.tile([128, 512], F32, tag="m", name="psv")
                ps = ps_full[:, 0:128]
                for kt in range(4):
                    nc.tensor.matmul(ps[:], xn[kt][:, tt * 128:(tt + 1) * 128],
                                     wv[kt][:], start=(kt == 0), stop=(kt == 3))
                nc.vector.tensor_tensor(
                    out=vsb[:, tt, :, 0:8],
                    in0=ps.rearrange("p (h d) -> p h d", d=8),
                    in1=bvb.rearrange("p (h d) -> p h d", d=8),
                    op=mybir.AluOpType.add)

            KQ_ORDER = [("k", 0), ("q", 0), ("k", 1), ("q", 1)]
            emit_kq_chunk(0, "k", 0)
            emit_kq_chunk(0, "q", 0)
            emit_v(0)
            emit_v(1)

            # ---- attention: 4 quads x (2 h-chunks x 8 f-blocks) ----
            hout = [singles.tile([128, L], BF16, name=f"ho{i}") for i in range(4)]

            def emit_exp(ps, use_act):
                if use_act:
                    wt = wtp.tile([128, 1024], BF16, tag="wt")
                    nc.scalar.activation(out=wt[:], in_=ps[:],
                                         func=mybir.ActivationFunctionType.Exp)
                else:
                    wti = wtp.tile([128, 1024], I16, tag="wt")
                    with nc.allow_low_precision(reason="schraudolph exp approx, validated"):
                        nc.vector.tensor_scalar(
                            out=wti[:], in0=ps[:], scalar1=EXP_A, scalar2=EXP_B,
                            op0=mybir.AluOpType.mult, op1=mybir.AluOpType.add)
                    wt = wti.bitcast(BF16)
                return wt

            rscr_t = rscr_d.tensor if hasattr(rscr_d, "tensor") else rscr_d

            def emit_norm(qd, h, a_sb, rb, mult_dve):
                """Recip the denominators of half h and scale a_sb into hout."""
                hc = slice(h * 512, (h + 1) * 512)
                dt = normp.tile([32, 64], BF16, tag="dt", name="dt")
                for j in range(4):
                    nc.sync.dma_start(
                        dt[8 * j:8 * j + 8, :],
                        a_sb[32 * j + 8:32 * j + 9, hc].rearrange("p (s f) -> p s f", f=64))
                rt = normp.tile([32, 64], BF16, tag="rt", name="rt")
                with nc.allow_low_precision(reason="bf16 softmax denom recip, validated"):
                    nc.vector.reciprocal(out=rt[:], in_=dt[:])
                if qd == 3:
                    # tail-latency path: SBUF-only broadcast via stream_shuffle
                    # (partition 32j+8 -> whole 32-band), no DRAM round trip
                    rq = normp.tile([128, 512], BF16, tag="rq", name="rq")
                    for j in range(4):
                        nc.sync.dma_start(
                            rq[32 * j + 8:32 * j + 9, :].rearrange("p (s f) -> p s f", f=64),
                            rt[8 * j:8 * j + 8, :])
                    nc.vector.stream_shuffle(rb[:, hc], rq[:], mask=[8] * 32)
                else:
                    for j in range(4):
                        hd_ = 4 * qd + j
                        nc.sync.dma_start(
                            rscr_d[hd_, h * 512:(h + 1) * 512].rearrange("(s f) -> s f", f=64),
                            rt[8 * j:8 * j + 8, :])
                    for j in range(4):
                        hd_ = 4 * qd + j
                        nc.sync.dma_start(
                            rb[32 * j:32 * j + 32, hc],
                            bass.AP(tensor=rscr_t, offset=hd_ * L + h * 512,
                                    ap=[[0, 32], [1, 512]]))
                mul_eng = nc.vector if mult_dve else nc.gpsimd
                mul_eng.tensor_tensor(out=hout[qd][:, hc], in0=a_sb[:, hc], in1=rb[:, hc],
                                      op=mybir.AluOpType.mult)

            def emit_outproj(mo, th, pool, tag):
                if tag == "sc":
                    ps_full = pool.tile([128, 1024], F32, tag=tag, name="pso")
                    ps = ps_full[:, 0:512]
                else:
                    ps = pool.tile([128, 512], F32, tag=tag, name="pso")
                for qdd in range(4):
                    nc.tensor.matmul(ps[:], wp[qdd][:, mo * 128:(mo + 1) * 128],
                                     hout[qdd][:, th * 512:(th + 1) * 512],
                                     start=(qdd == 0), stop=(qdd == 3))
                ot = outp.tile([128, 512], F32, tag="ot")
                nc.scalar.activation(out=ot[:], in_=ps[:],
                                     func=mybir.ActivationFunctionType.Copy)
                nc.gpsimd.dma_start(out_d[mo * 128:(mo + 1) * 128, th * 512:(th + 1) * 512],
                                    ot[:])

            av_t = {}
            pends = []  # fifo of dicts, AV runs 2 cycles behind its scores

            def process_pend(p):
                pqd, pfb, ph = p["qd"], p["fb"], p["h"]
                if pfb == 0:
                    av_t[ph] = psA.tile([128, 512], F32, tag="av", name="av")
                av = av_t[ph]
                hd0 = 4 * pqd
                pwtA, pwtB = p["wtA"], p["wtB"]
                nc.tensor.matmul(av[0:32, :], vsb[:, pfb, hd0 + 0, :], pwtA[:, 0:512],
                                 start=(pfb == 0), stop=(pfb == 7), tile_position=(0, 0))
                nc.tensor.matmul(av[32:64, :], vsb[:, pfb, hd0 + 1, :], pwtA[:, 512:1024],
                                 start=(pfb == 0), stop=(pfb == 7), tile_position=(0, 32))
                nc.tensor.matmul(av[64:96, :], vsb[:, pfb, hd0 + 2, :], pwtB[:, 0:512],
                                 start=(pfb == 0), stop=(pfb == 7), tile_position=(0, 64))
                nc.tensor.matmul(av[96:128, :], vsb[:, pfb, hd0 + 3, :], pwtB[:, 512:1024],
                                 start=(pfb == 0), stop=(pfb == 7), tile_position=(0, 96))
                if pfb == 7:
                    # dump av(h) -> a_sb chunk (ACT for h0, DVE for h1); this
                    # is emitted before this cycle's exps so it lands early in
                    # the engine queue
                    dst = p["a_sb"][:, ph * 512:(ph + 1) * 512]
                    if ph == 0:
                        nc.scalar.activation(out=dst, in_=av[:],
                                             func=mybir.ActivationFunctionType.Copy)
                    else:
                        nc.vector.tensor_copy(dst, av[:])
                    emit_norm(pqd, ph, p["a_sb"], p["rb"],
                              mult_dve=(pqd == 3 and ph == 1))

            for qd in range(4):
                if qd == 1:
                    for ct in range(4):
                        nc.gpsimd.dma_start(wp[ct][:], wp_d[ct * 128:(ct + 1) * 128, :])
                a_sb = normp.tile([128, 1024], BF16, tag="asb")
                rb = normp.tile([128, 1024], BF16, tag="rb")

                for c in range(16):
                    h, fb = divmod(c, 8)
                    fc = slice(fb * 128, (fb + 1) * 128)
                    hc = slice(h * 512, (h + 1) * 512)
                    A = psS.tile([128, 1024], F32, tag="sc", name="ps_scA")
                    B_ = psS.tile([128, 1024], F32, tag="sc", name="ps_scB")
                    nc.tensor.matmul(A[:, 0:512], kT[qd][0:32, fc], qT[qd][0:32, hc],
                                     tile_position=(0, 0))
                    nc.tensor.matmul(A[:, 512:1024], kT[qd][32:64, fc], qT[qd][32:64, hc],
                                     tile_position=(32, 0))
                    nc.tensor.matmul(B_[:, 0:512], kT[qd][64:96, fc], qT[qd][64:96, hc],
                                     tile_position=(64, 0))
                    nc.tensor.matmul(B_[:, 512:1024], kT[qd][96:128, fc], qT[qd][96:128, hc],
                                     tile_position=(96, 0))

                    if len(pends) >= 2:
                        process_pend(pends.pop(0))

                    both_act = c in ACT_BOTH_CYCLES
                    wtA = emit_exp(A, True)
                    wtB = emit_exp(B_, both_act)
                    pends.append({"wtA": wtA, "wtB": wtB, "fb": fb, "h": h,
                                  "qd": qd, "a_sb": a_sb, "rb": rb})

                    # interleaved projections
                    if qd == 0:
                        if c == 0:
                            emit_v(2)
                        elif c == 1:
                            emit_kq_chunk(0, "k", 1)
                            emit_v(3)
                        elif c == 2:
                            emit_kq_chunk(0, "q", 1)
                            emit_v(4)
                        elif c in (3, 4, 5):
                            emit_v(c + 2)
                    if qd < 3 and c in (9, 11, 13, 15):
                        which, nh_ = KQ_ORDER[(c - 9) // 2]
                        emit_kq_chunk(qd + 1, which, nh_)
                    if qd == 3 and c in (13, 15):
                        mo = (c - 13) // 2
                        emit_outproj(mo, 0, psM, "m")
            while pends:
                process_pend(pends.pop(0))

            # ---- output projection remainder (rotate pools for pipelining) ----
            rem = [(2, 0), (3, 0), (0, 1), (1, 1), (2, 1), (3, 1)]
            pools = [(psS, "sc"), (psM, "m"), (psA, "av")]
            for i, (mo, th) in enumerate(rem):
                pool, tag = pools[i % 3]
                emit_outproj(mo, th, pool, tag)

    nc.compile()
    return nc


def _prep_inputs(x, gamma, beta, w_qkv, b_qkv, w_proj, b_proj, num_heads):
    """Shard and lay out inputs for the 8 cores."""
    nh = int(num_heads)
    hd = C // nh
    scale = (3 * nh) ** (-0.5)
    wq_full, wk_full, wv_full = w_qkv[0:C], w_qkv[C:2 * C], w_qkv[2 * C:3 * C]
    bq_full, bk_full, bv_full = b_qkv[0:C], b_qkv[C:2 * C], b_qkv[2 * C:3 * C]

    bf16 = ml_dtypes.bfloat16
    in_maps = []
    for core in range(N_CORES):
        bi = core // 4
        g = core % 4
        heads = list(range(HEADS_PER_CORE * g, HEADS_PER_CORE * (g + 1)))
        x_own = x[bi]
        x_oth = x[1 - bi]
        x2 = np.concatenate([x_own, x_oth], axis=1).astype(bf16)

        wqT = np.zeros((C, 512), np.float32)
        wkT = np.zeros((C, 512), np.float32)
        bq = np.zeros(512, np.float32)
        bk = np.zeros(512, np.float32)
        wvT = np.zeros((C, 128), np.float32)
        bv = np.zeros(128, np.float32)
        wpT = np.zeros((512, C), np.float32)
        for jl, h in enumerate(heads):
            for d in range(nh):
                ch = d * hd + h
                wqT[:, 32 * jl + d] = wq_full[ch] * scale
                wkT[:, 32 * jl + d] = wk_full[ch]
                bq[32 * jl + d] = bq_full[ch] * scale
                bk[32 * jl + d] = bk_full[ch]
                wvT[:, 8 * jl + d] = wv_full[ch]
                bv[8 * jl + d] = bv_full[ch]
                wpT[32 * jl + d, :] = w_proj[:, ch]

        in_maps.append({
            "x2": np.ascontiguousarray(x2),
            "gamma": np.ascontiguousarray(gamma.astype(np.float32)),
            "beta": np.ascontiguousarray(beta.astype(np.float32)),
            "wqT": np.ascontiguousarray(wqT.astype(bf16)),
            "wkT": np.ascontiguousarray(wkT.astype(bf16)),
            "wvT": np.ascontiguousarray(wvT.astype(bf16)),
            "bq": bq, "bk": bk, "bv": bv,
            "wpT": np.ascontiguousarray(wpT.astype(bf16)),
        })
    return in_maps


def kernel(x, gamma, beta, w_qkv, b_qkv, w_proj, b_proj, num_heads, _trace=False):
    x = np.asarray(x, dtype=np.float32)
    gamma = np.asarray(gamma, np.float32)
    beta = np.asarray(beta, np.float32)
    w_qkv = np.asarray(w_qkv, np.float32)
    b_qkv = np.asarray(b_qkv, np.float32)
    w_proj = np.asarray(w_proj, np.float32)
    b_proj = np.asarray(b_proj, np.float32)

    if "nc" not in _CACHE:
        _CACHE["nc"] = _build_nc()
    nc = _CACHE["nc"]
    in_maps = _prep_inputs(x, gamma, beta, w_qkv, b_qkv, w_proj, b_proj, num_heads)
    res = run_bass_kernel_spmd(nc, in_maps, core_ids=list(range(N_CORES)), trace=_trace)
    _CACHE["last_result"] = res

    out = np.zeros((B, C, L), np.float32)
    for bi in range(B):
        acc = x[bi] + b_proj[:, None]
        for g in range(4):
            acc = acc + np.asarray(res.results[bi * 4 + g]["out"])
        out[bi] = acc
    return out
